# revision 12
# baseline (speedup 1.0000x reference)
"""MiniMax-M2 sparse MoE block on 8 Trainium2 NeuronCores.

Strategy (expert-parallel, fp8-weight stage A):
  - Host: router (fp64 gating + biased top-2), token dispatch, weight
    quantization/layout prep, final weighted combine.  All tiny next to
    the expert MLPs.
  - Device: each of the 8 cores owns 2 of the 16 experts (slot 0 = one
    of the 8 busiest experts, slot 1 = one of the 8 least busy, so the
    two capacity paddings C0 >= C1 stay tight) and runs the SwiGLU MLP
    in transposed token layout:
        h1T[I,C] = sum_k w1[k,I].T @ xT[k,C]      (k = 128-row H chunks)
        heT      = silu(h1T) * h3T
        yT[H,C]  = sum_i w2b[i,H].T @ heT[i,C]    (i = 128-row I chunks)
  - Precision: w1/w3 are stored as fp8 e3m4 scaled by a per-expert
    power of two s1 (absmax -> ~14); the inverse scale is folded into
    the bf16 token tile (exact, exponent-only), so PSUM h1/h3 come out
    in true scale with no device-side descaling.  w2 and the moving
    operands stay bf16 (measured end-to-end rel err 1.8e-2 < 2e-2 gate,
    vs 4.5e-3 all-bf16).  This cuts weight DMA from 25.2 MB to 16.8 MB
    per core; mixed fp8(stationary) x bf16(moving) matmuls are
    supported by the PE.
  - DMA: all loads ride the sync HWDGE queue in exact consumption
    order (x first so the first matmul starts early), packed into
    ~0.5-2 MB transfers to amortize descriptor generation.  Output
    stores ride the scalar HWDGE ring so they never block loads.

Shapes hardcoded per the problem spec: T=1024, H=2048, I=1024, E=16,
top-2, fp32 I/O.
"""

import os

import numpy as np
import ml_dtypes

T, H, I, E, TOPK = 1024, 2048, 1024, 16, 2
N_CORES = 8
E_LOC = E // N_CORES  # expert slots per core
P = 128               # partition size
KH = H // P           # 16 contraction chunks over H (stage A)
MI = I // P           # 8 output blocks over I (stage A) / contraction (stage B)
MH = H // P           # 16 output blocks over H (stage B)

# stage-A weight dtype: "e3" (fp8 e3m4, default) | "bf16" (fallback)
W13_PREC = os.environ.get("MOE_W13", "e3")
TRACE = os.environ.get("MOE_TRACE", "0") == "1"

LAST_RESULTS = None  # BassKernelResults of the last run (for test harness)
_RUN_IDX = 0

_BUILD_CACHE: dict = {}


def _slot_geom(C):
    """Token chunking for one slot: NC chunks of nb tokens (nb <= 256 so
    the packed h1|h3 PSUM tile [P, 2*nb] fp32 fits one 2 KiB bank)."""
    NC = (C + 255) // 256
    nb = C // NC
    assert C % NC == 0 and nb <= 256, (C, NC, nb)
    return NC, nb


def _build(Cs, w13_prec):
    """Build + lower the per-core Bass program (same SPMD program on all
    cores; per-core data differs via in_maps).  Cs = (C0, C1) per-slot
    token capacities."""
    key = (Cs, w13_prec)
    if key in _BUILD_CACHE:
        return _BUILD_CACHE[key]

    import concourse.bacc as bacc
    import concourse.tile as tile
    import concourse.mybir as mybir
    from concourse.bass import ts, ds

    dt_w13 = mybir.dt.float8e3 if w13_prec == "e3" else mybir.dt.bfloat16
    bf16 = mybir.dt.bfloat16
    f32 = mybir.dt.float32

    geom = [_slot_geom(C) for C in Cs]
    resident = max(Cs) <= 512  # everything fits SBUF comfortably

    nc = bacc.Bacc("TRN2", target_bir_lowering=False, debug=False,
                   num_devices=N_CORES)

    # xT pre-tiled on host: xT{e}[p, kb*C+c] = (x_gathered[e]/s1_e)[kb*128+p, c]
    xT = [nc.dram_tensor(f"xT{e}", [P, KH * Cs[e]], bf16,
                         kind="ExternalInput") for e in range(E_LOC)]
    # w13 packed per kb chunk: w13[e, p, kb*2I + i]       = (w1[e]*s1)[kb*128+p, i]
    #                          w13[e, p, kb*2I + I + i]   = (w3[e]*s1)[kb*128+p, i]
    w13 = nc.dram_tensor("w13", [E_LOC, P, KH * 2 * I], dt_w13,
                         kind="ExternalInput")
    # w2 pre-blocked: w2t[e, r, hb*I + ib*128 + c] = w2[e, ib*128+r, hb*128+c]
    w2t = nc.dram_tensor("w2t", [E_LOC, P, MH * I], bf16, kind="ExternalInput")
    # yT tiled: yT{e}[p, hb*C+c] = y_e[hb*128+p, c]; host un-tiles.  bf16:
    # host combine runs in fp64 and the outputs already carry bf16-compute
    # noise, so this only adds one rounding while halving store bytes.
    yT = [nc.dram_tensor(f"yT{e}", [P, MH * Cs[e]], bf16,
                         kind="ExternalOutput") for e in range(E_LOC)]

    XS = 2            # x load split (smaller first transfer)
    W2G = 4           # hb blocks per w2 DMA
    OG = 4            # output store DMA groups per expert
    # w13 chunking per expert: slot 0's first chunks are single-kb so the
    # completion semaphore of the matmul-gating first chunk fires ~1us
    # earlier (receipt latency scales with nothing, transfer with size).
    W13_CHUNKS = {0: [1, 1] + [2] * ((KH - 2) // 2), 1: [2] * (KH // 2)}

    # Residency: when everything fits (the practical case), keep all
    # weight tiles live so DMAs stream with no pool recycling.  The
    # big-C fallback streams w13/w2 chunks through smaller rings.
    NCtot = sum(nc_ for nc_, _ in geom)
    NW13 = sum(len(W13_CHUNKS[e]) for e in range(E_LOC))
    WP_BUFS = NW13 if resident else len(W13_CHUNKS[0]) + 2
    W2P_BUFS = E_LOC * (MH // W2G) if resident else 3
    YP_BUFS = E_LOC * OG if resident else 1
    HP_BUFS = MI * NCtot

    with tile.TileContext(nc) as tc:
        with (
            tc.tile_pool(name="xp", bufs=E_LOC) as xp,
            tc.tile_pool(name="wp", bufs=WP_BUFS) as wp,
            tc.tile_pool(name="w2p", bufs=W2P_BUFS) as w2p,
            tc.tile_pool(name="hp", bufs=HP_BUFS) as hp,
            tc.tile_pool(name="sp", bufs=3) as sp,
            tc.tile_pool(name="yp", bufs=YP_BUFS) as yp,
            tc.tile_pool(name="pk", bufs=MI, space="PSUM") as pk,
        ):
            # ---- loads: weights stream on the sync HWDGE queue; token
            # tiles ride the scalar HWDGE ring in parallel so the sync
            # queue's limited head-of-line issue window goes entirely to
            # the weight chunks the first matmuls are gated on.
            xbig = [xp.tile([P, KH * Cs[e]], bf16, tag="xt", name=f"xt{e}")
                    for e in range(E_LOC)]

            def load_x(e, nsplit):
                step = (KH * Cs[e]) // nsplit
                for q in range(nsplit):
                    nc.scalar.dma_start(xbig[e][:, ds(q * step, step)],
                                        xT[e][:, ds(q * step, step)])

            # per (expert, kb): (tile, offset of kb within tile)
            w13t_all = [[None] * KH for _ in range(E_LOC)]

            def load_w13(e):
                kb0 = 0
                for g, nkb in enumerate(W13_CHUNKS[e]):
                    t = wp.tile([P, nkb * 2 * I], dt_w13, tag="w13",
                                name=f"w13_{e}_{g}")
                    nc.sync.dma_start(t[:], w13[e, :, ds(kb0 * 2 * I,
                                                         nkb * 2 * I)])
                    for j in range(nkb):
                        w13t_all[e][kb0 + j] = (t, j * 2 * I)
                    kb0 += nkb

            w2t_all = [[None] * (MH // W2G) for _ in range(E_LOC)]

            def load_w2(e):
                for g in range(MH // W2G):
                    t = w2p.tile([P, W2G * I], bf16, tag="w2",
                                 name=f"w2_{e}_{g}")
                    nc.sync.dma_start(t[:], w2t[e, :, ds(g * W2G * I,
                                                         W2G * I)])
                    w2t_all[e][g] = t

            load_x(0, XS)
            load_x(1, XS)
            for e in range(E_LOC):
                load_w13(e)
            for e in range(E_LOC):
                load_w2(e)

            def w13_ap(e, kb, which, ib):
                # which: 0 = w1, 1 = w3 ; returns [P, 128] stationary slice
                t, base = w13t_all[e][kb]
                return t[:, ds(base + which * I + ib * P, P)]

            def w2_ap(e, hb, ib):
                t = w2t_all[e][hb // W2G]
                off = (hb % W2G) * I + ib * P
                return t[:, ds(off, P)]

            het_all = [None] * E_LOC

            def stage_A(e):
                C = Cs[e]
                NC, nb = geom[e]
                # kb-outer: all MI h1/h3 blocks accumulate at once, so the
                # PE consumes each weight chunk the moment it lands.  h1
                # and h3 for one ib share a single PSUM bank: p13[:, :nb]
                # is h1, p13[:, nb:] is h3.
                het = [[None] * MI for _ in range(NC)]
                for cb in range(NC):
                    p13 = [pk.tile([P, 2 * nb], f32, tag="pk",
                                   name=f"p13_{e}_{cb}_{ib}")
                           for ib in range(MI)]
                    # One accumulation group per bank: start=True only on
                    # the bank's first matmul (h1,kb=0) -- it clears
                    # has_written for the whole bank; h3's kb=0 then lands
                    # by per-element overwrite-where-unwritten.
                    for kb in range(KH - 1):
                        rhs = xbig[e][:, ds(kb * C + cb * nb, nb)]
                        for ib in range(MI):
                            nc.tensor.matmul(
                                p13[ib][:, ds(0, nb)], w13_ap(e, kb, 0, ib),
                                rhs, start=(kb == 0), stop=False)
                        for ib in range(MI):
                            nc.tensor.matmul(
                                p13[ib][:, ds(nb, nb)], w13_ap(e, kb, 1, ib),
                                rhs, start=False, stop=False)
                    # Last chunk pairwise per-ib so each bank closes (and
                    # its silu chain + PSUM slot release starts) early.
                    kb = KH - 1
                    rhs = xbig[e][:, ds(kb * C + cb * nb, nb)]
                    for ib in range(MI):
                        nc.tensor.matmul(
                            p13[ib][:, ds(0, nb)], w13_ap(e, kb, 0, ib),
                            rhs, start=False, stop=False)
                        nc.tensor.matmul(
                            p13[ib][:, ds(nb, nb)], w13_ap(e, kb, 1, ib),
                            rhs, start=False, stop=True)
                        # silu(h1)=h1*sigmoid(h1); no Silu LUT in CoreSim
                        s = sp.tile([P, nb], f32, tag="s",
                                    name=f"s_{e}_{cb}_{ib}")
                        nc.scalar.activation(
                            s[:], p13[ib][:, ds(0, nb)],
                            mybir.ActivationFunctionType.Sigmoid)
                        u = sp.tile([P, nb], f32, tag="u",
                                    name=f"u_{e}_{cb}_{ib}")
                        nc.vector.tensor_mul(u[:], s[:], p13[ib][:, ds(0, nb)])
                        h = hp.tile([P, nb], bf16, tag="he",
                                    name=f"he_{e}_{cb}_{ib}")
                        nc.vector.tensor_mul(h[:], u[:], p13[ib][:, ds(nb, nb)])
                        het[cb][ib] = h
                het_all[e] = het

            def stage_B(e):
                C = Cs[e]
                NC, nb = geom[e]
                het = het_all[e]
                GH = MH // OG
                yst = None
                for hb in range(MH):
                    # group-local staging tile: the store DMA of group g
                    # and the copies of group g+1 touch different tiles,
                    # so no WAR dependency couples them.
                    if resident and hb % GH == 0:
                        yst = yp.tile([P, GH * C], bf16, tag="yst",
                                      name=f"yst_{e}_{hb // GH}")
                    for cb in range(NC):
                        py = pk.tile([P, nb], f32, tag="pk",
                                     name=f"py_{e}_{hb}_{cb}")
                        for ib in range(MI):
                            nc.tensor.matmul(
                                py[:], w2_ap(e, hb, ib), het[cb][ib][:],
                                start=(ib == 0), stop=(ib == MI - 1))
                        if resident:
                            nc.vector.tensor_copy(
                                yst[:, ds((hb % GH) * C + cb * nb, nb)], py[:])
                        else:
                            yo = yp.tile([P, nb], bf16, tag="yo",
                                         name=f"yo_{e}_{hb}_{cb}")
                            nc.vector.tensor_copy(yo[:], py[:])
                            nc.gpsimd.dma_start(
                                yT[e][:, ds(hb * C + cb * nb, nb)], yo[:])
                    if resident and (hb + 1) % GH == 0:
                        g0 = (hb + 1 - GH) * C
                        nc.gpsimd.dma_start(
                            yT[e][:, ds(g0, GH * C)], yst[:])

            for e in range(E_LOC):
                stage_A(e)
            for e in range(E_LOC):
                stage_B(e)

    nc.compile()
    _BUILD_CACHE[key] = nc
    return nc


def _route(x: np.ndarray, gate_w: np.ndarray, bias: np.ndarray):
    """Reference-equivalent router, done in fp64 for tie stability.
    Returns per-expert token index lists and combine weights."""
    logits = x.astype(np.float64) @ gate_w.astype(np.float64).T      # [T, E]
    m = logits.max(axis=1, keepdims=True)
    p = np.exp(logits - m)
    scores = p / p.sum(axis=1, keepdims=True)                        # [T, E]
    biased = scores + bias.astype(np.float64)[None, :]
    # top-2, ties to lower index (matches jax.lax.top_k)
    idx = np.argsort(-biased, axis=1, kind="stable")[:, :TOPK]       # [T, 2]
    tw = np.take_along_axis(scores, idx, axis=1)
    tw = tw / tw.sum(axis=1, keepdims=True)                          # [T, 2]

    flat_e = idx.ravel()
    flat_t = np.repeat(np.arange(T), TOPK)
    flat_w = tw.ravel()
    order = np.argsort(flat_e, kind="stable")
    fe, ft, fw = flat_e[order], flat_t[order], flat_w[order]
    starts = np.searchsorted(fe, np.arange(E + 1))
    tok = [ft[starts[e]:starts[e + 1]] for e in range(E)]
    wgt = [fw[starts[e]:starts[e + 1]] for e in range(E)]
    return tok, wgt


def _round_up(n, m):
    return m * ((n + m - 1) // m)


def kernel(hidden_states, gate_w, bias, w1, w3, w2):
    global LAST_RESULTS
    from concourse.bass_utils import run_bass_kernel_spmd

    x = np.asarray(hidden_states, dtype=np.float32)
    gate_w = np.asarray(gate_w, dtype=np.float32)
    bias = np.asarray(bias, dtype=np.float32)
    w1 = np.asarray(w1, dtype=np.float32)
    w3 = np.asarray(w3, dtype=np.float32)
    w2 = np.asarray(w2, dtype=np.float32)

    tok, wgt = _route(x, gate_w, bias)
    counts = np.array([len(t) for t in tok])

    # Slot assignment: slot 0 = the 8 busiest experts, slot 1 = the 8
    # least busy; core c gets (big[c], small[c]).  Capacities per slot.
    order = np.argsort(-counts, kind="stable")
    slot_experts = [list(order[:N_CORES]), list(order[N_CORES:][::-1])]

    def cap(n):
        c = max(32, _round_up(n, 4))
        if c > 256:  # imbalanced routing: NC chunks of nb <= 256
            NCc = (c + 255) // 256
            nbc = _round_up(-(-c // NCc), 8)
            c = NCc * nbc
        return c

    Cs = tuple(int(cap(max(int(counts[e]) for e in slot_experts[s])))
               for s in range(E_LOC))

    w13_prec = W13_PREC
    e3 = ml_dtypes.float8_e3m4
    xt_f32 = x.T  # [H, T]

    nc = _build(Cs, w13_prec)

    in_maps = [dict() for _ in range(N_CORES)]
    scale1 = np.zeros(E)
    for s in range(E_LOC):
        C = Cs[s]
        for c in range(N_CORES):
            e = slot_experts[s][c]
            # per-expert power-of-2 scale for w1/w3 -> e3m4 range (~14)
            if w13_prec == "e3":
                amax = max(np.abs(w1[e]).max(), np.abs(w3[e]).max())
                s1 = 2.0 ** np.floor(np.log2(14.0 / amax))
            else:
                s1 = 1.0
            scale1[e] = s1

            # tokens, pre-tiled + descaled: [P, KH*C]
            xe = np.zeros((H, C), np.float32)
            n = len(tok[e])
            if n:
                xe[:, :n] = xt_f32[:, tok[e]]
            xe *= (1.0 / s1)
            in_maps[c][f"xT{s}"] = np.ascontiguousarray(
                xe.reshape(KH, P, C).transpose(1, 0, 2)
                .reshape(P, KH * C)).astype(ml_dtypes.bfloat16)

            # w13 packed per kb: [P, KH*2I]
            w13e = np.empty((P, KH, 2, I), np.float32)
            w13e[:, :, 0, :] = (w1[e] * s1).reshape(KH, P, I).transpose(1, 0, 2)
            w13e[:, :, 1, :] = (w3[e] * s1).reshape(KH, P, I).transpose(1, 0, 2)
            wdt = e3 if w13_prec == "e3" else ml_dtypes.bfloat16
            in_maps[c].setdefault("w13", np.empty((E_LOC, P, KH * 2 * I), wdt))
            in_maps[c]["w13"][s] = w13e.reshape(P, KH * 2 * I).astype(wdt)

            # w2 pre-blocked: w2t[r, hb*I + ib*128 + c2] = w2[e, ib*128+r, hb*128+c2]
            w2e = (w2[e].reshape(MI, P, MH, P).transpose(1, 2, 0, 3)
                   .reshape(P, MH * I))
            in_maps[c].setdefault("w2t", np.empty((E_LOC, P, MH * I),
                                                  ml_dtypes.bfloat16))
            in_maps[c]["w2t"][s] = w2e.astype(ml_dtypes.bfloat16)

    for c in range(N_CORES):
        for k in list(in_maps[c]):
            in_maps[c][k] = np.ascontiguousarray(in_maps[c][k])

    kwargs = {}
    if TRACE:
        kwargs.update(trace=True, trace_cores=[0])
        if os.environ.get("MOE_TMPDIR"):
            global _RUN_IDX
            _RUN_IDX += 1
            td = os.path.join(os.environ["MOE_TMPDIR"], f"r{_RUN_IDX}")
            os.makedirs(td, exist_ok=True)
            kwargs["tmpdir"] = td
    res = run_bass_kernel_spmd(nc, in_maps, core_ids=list(range(N_CORES)),
                               **kwargs)
    LAST_RESULTS = res

    out = np.zeros((T, H), dtype=np.float64)
    for s in range(E_LOC):
        C = Cs[s]
        for c in range(N_CORES):
            e = slot_experts[s][c]
            n = len(tok[e])
            if not n:
                continue
            yTt = np.asarray(res.results[c][f"yT{s}"])   # [P, MH*C] bf16 tiled
            # un-tile: [P, MH, C] -> [MH, P, C] -> [H, C]
            y_full = yTt.reshape(P, MH, C).transpose(1, 0, 2).reshape(H, C)
            y = y_full[:, :n].T.astype(np.float64)       # [n, H]
            out[tok[e]] += wgt[e][:, None] * y
    return out.astype(np.float32)
